# revision 1
# baseline (speedup 1.0000x reference)
"""CapsuleRewardHead Trainium2 kernel (8-core data parallel).

Math (per batch row b):
    primary = x @ W + b_lin                    [B, 128]  (128 = 8 caps x 16 dim)
    u_hat[b,o,i,j] = sum_c primary[b,i,c] * out_caps[o,i,c,j]
    3 rounds of dynamic routing over N=32 capsule pairs (o,i), D=16
    out[b] = |squash(s_final)|

Device strategy per core (2048 batch rows):
  - host: transpose x shard -> xt [4096, 2048] fp32 so the hidden dim lands on
    SBUF partitions (PE contracts over partitions); replicate small params.
  - stream xt in batch-slices of SUP_B cols, casting fp32->bf16 in the DMA
    (SWDGE inline cast; HBM read of fp32 is the roofline term).
  - MM1 (PE): primaryT[ic, b] += W[h,ic].T @ xT[h, b] accumulated over 32
    h-chunks into PSUM; drain PSUM->SBUF on ACT with per-partition bias add
    (the Linear bias) and bf16 cast.
  - MM2 (PE): u_hat[b, (o,i,j)] via block-diagonal capsule matrices:
    lhsT = primaryT chunk [128ic, 128b] (stationary), rhs = caps_bd[o]
    [128ic, 128ij] -> PSUM [128b, 512] directly in routing-friendly layout.
  - routing: grouped free-axis reduces on DVE, exp/sqrt/copies on ACT,
    big broadcast multiplies on GPSIMD, all per 128-row chunk.
"""

import os

import numpy as np
import ml_dtypes

B = 16384
HIDDEN = 4096
NUM_OBJ = 4
NUM_CAPS = 8
CAP_DIM = 16
N_ROUTE = 32  # NUM_OBJ * NUM_CAPS
N_CORES = 8

# exposed for test.py; set after each run when BASS_TRACE=1
LAST_EXEC_TIME_NS = None

BF16 = ml_dtypes.bfloat16


def _bcast_inner(ap, n):
    """Append a stride-0 dim of size n (broadcast along a new inner axis)."""
    import concourse.bass as bass

    return bass.AP(tensor=ap.tensor, offset=ap.offset, ap=[*ap.ap, [0, n]])


def _bcast_mid(ap2d, n):
    """[P, d] -> [P, n, d] with stride-0 middle dim."""
    import concourse.bass as bass

    return bass.AP(
        tensor=ap2d.tensor, offset=ap2d.offset, ap=[ap2d.ap[0], [0, n], ap2d.ap[1]]
    )


def build_bass(hidden=HIDDEN, b_sh=B // N_CORES, sup_b=256, mul_engine="gpsimd"):
    """Emit the per-core Bass program. Returns the compiled Bacc object."""
    import concourse.bass as bass
    import concourse.tile as tile
    from concourse import bacc, mybir

    NH = hidden // 128  # h-chunks
    NSUP = b_sh // sup_b  # batch super-tiles (one DMA each)
    NCH = sup_b // 128  # 128-row routing chunks per super
    dt = mybir.dt

    nc = bacc.Bacc("TRN2", target_bir_lowering=False, debug=False, num_devices=N_CORES)

    xt_ap = nc.dram_tensor("xt", [hidden, b_sh], dt.float32, kind="ExternalInput").ap()
    w_ap = nc.dram_tensor("w", [NH, 128, 128], dt.bfloat16, kind="ExternalInput").ap()
    caps_ap = nc.dram_tensor(
        "caps", [NUM_OBJ, 128, 128], dt.bfloat16, kind="ExternalInput"
    ).ap()
    bias_ap = nc.dram_tensor("bias", [128, 1], dt.float32, kind="ExternalInput").ap()
    out_ap = nc.dram_tensor("out", [b_sh], dt.float32, kind="ExternalOutput").ap()

    AX = mybir.AxisListType
    OP = mybir.AluOpType
    AF = mybir.ActivationFunctionType

    with tile.TileContext(nc) as tc:
        with (
            tc.tile_pool(name="singles", bufs=1) as singles,
            tc.tile_pool(name="xs", bufs=3) as xs_pool,
            tc.tile_pool(name="primt", bufs=2) as primt_pool,
            tc.tile_pool(name="uh", bufs=3) as uh_pool,
            tc.tile_pool(name="tmp", bufs=3) as tmp_pool,
            tc.tile_pool(name="sm", bufs=4) as sm_pool,
            tc.tile_pool(name="tiny", bufs=16) as tiny_pool,
            tc.tile_pool(name="psum_p", bufs=2, space="PSUM") as psp_pool,
            tc.tile_pool(name="psum_u", bufs=3, space="PSUM") as psu_pool,
        ):
            # --- resident parameters ---
            w_sb = singles.tile([128, NH, 128], dt.bfloat16)
            nc.sync.dma_start(out=w_sb[:], in_=w_ap.rearrange("h p f -> p h f"))
            caps_sb = singles.tile([128, NUM_OBJ, 128], dt.bfloat16)
            nc.sync.dma_start(out=caps_sb[:], in_=caps_ap.rearrange("o p f -> p o f"))
            bias_sb = singles.tile([128, 1], dt.float32)
            nc.sync.dma_start(out=bias_sb[:], in_=bias_ap[:, :])
            out_sb = singles.tile([128, NSUP * NCH], dt.float32)

            xt_v = xt_ap.rearrange("(hc p) b -> p hc b", p=128)

            mul_eng = getattr(nc, mul_engine)

            for s in range(NSUP):
                # batch-slice load: [128p, NH, sup_b] bf16, cast in the DMA
                xs = xs_pool.tile([128, NH, sup_b], dt.bfloat16)
                nc.gpsimd.dma_start(
                    out=xs[:], in_=xt_v[:, :, s * sup_b : (s + 1) * sup_b]
                )

                # MM1: primaryT[ic, b] accumulated over h-chunks
                psp = psp_pool.tile([128, sup_b], dt.float32)
                for h in range(NH):
                    nc.tensor.matmul(
                        psp[:],
                        w_sb[:, h, :],
                        xs[:, h, :],
                        start=(h == 0),
                        stop=(h == NH - 1),
                    )

                # drain with bias add + bf16 cast (ACT)
                primt = primt_pool.tile([128, sup_b], dt.bfloat16)
                nc.scalar.activation(
                    primt[:], psp[:], AF.Identity, bias=bias_sb[:], scale=1.0
                )

                for c in range(NCH):
                    ch = s * NCH + c
                    # MM2: u_hat [128b, (o,i,j)=512] in PSUM
                    psu = psu_pool.tile([128, 4 * 128], dt.float32)
                    for o in range(NUM_OBJ):
                        nc.tensor.matmul(
                            psu[:, o * 128 : (o + 1) * 128],
                            primt[:, c * 128 : (c + 1) * 128],
                            caps_sb[:, o, :],
                            start=True,
                            stop=True,
                        )

                    # SBUF copy (bf16) for routing
                    uh = uh_pool.tile([128, N_ROUTE * CAP_DIM], dt.bfloat16)
                    nc.scalar.copy(uh[:], psu[:])
                    uh_nd = uh.rearrange("p (n d) -> p n d", n=N_ROUTE)
                    uh_dn = uh.rearrange("p (n d) -> p d n", n=N_ROUTE)

                    b_log = None  # routing logits [128, 32] fp32
                    for r in range(3):
                        if r == 0:
                            # c uniform = 1/32: t = sum_n u
                            t = sm_pool.tile([128, CAP_DIM], dt.float32)
                            nc.vector.tensor_reduce(t[:], uh_dn, axis=AX.X, op=OP.add)
                            s_v = sm_pool.tile([128, CAP_DIM], dt.float32)
                            nc.vector.tensor_scalar_mul(s_v[:], t[:], 1.0 / N_ROUTE)
                        else:
                            # softmax(b) without normalization; fold 1/se into s
                            negmx = tiny_pool.tile([128, 1], dt.float32)
                            nc.vector.tensor_reduce(
                                negmx[:], b_log[:], axis=AX.X, op=OP.max, negate=True
                            )
                            e = sm_pool.tile([128, N_ROUTE], dt.bfloat16)
                            nc.scalar.activation(
                                e[:], b_log[:], AF.Exp, bias=negmx[:], scale=1.0
                            )
                            se = tiny_pool.tile([128, 1], dt.float32)
                            nc.vector.tensor_reduce(se[:], e[:], axis=AX.X, op=OP.add)
                            rse = tiny_pool.tile([128, 1], dt.float32)
                            nc.vector.reciprocal(rse[:], se[:])
                            # t[d] = sum_n e_n * u[n,d]
                            tmp = tmp_pool.tile([128, N_ROUTE, CAP_DIM], dt.bfloat16)
                            mul_eng.tensor_tensor(
                                tmp[:], uh_nd, _bcast_inner(e[:], CAP_DIM), op=OP.mult
                            )
                            t = sm_pool.tile([128, CAP_DIM], dt.float32)
                            nc.vector.tensor_reduce(
                                t[:],
                                tmp.rearrange("p n d -> p d n"),
                                axis=AX.X,
                                op=OP.add,
                            )
                            s_v = sm_pool.tile([128, CAP_DIM], dt.float32)
                            nc.vector.tensor_scalar_mul(s_v[:], t[:], rse[:])

                        # squash scalars: nn = |s|^2, n1 = |s|,
                        # g = nn / ((1+nn)(n1+1e-8))   (v = g*s)
                        scr = sm_pool.tile([128, CAP_DIM], dt.float32)
                        nn = tiny_pool.tile([128, 1], dt.float32)
                        nc.vector.scalar_tensor_tensor(
                            scr[:], s_v[:], 1.0, s_v[:],
                            op0=OP.mult, op1=OP.mult, accum_out=nn[:],
                        )
                        n1 = tiny_pool.tile([128, 1], dt.float32)
                        nc.scalar.sqrt(n1[:], nn[:])

                        if r < 2:
                            d1 = tiny_pool.tile([128, 1], dt.float32)
                            nc.vector.tensor_scalar_add(d1[:], nn[:], 1.0)
                            d2 = tiny_pool.tile([128, 1], dt.float32)
                            nc.vector.tensor_scalar_add(d2[:], n1[:], 1e-8)
                            d3 = tiny_pool.tile([128, 1], dt.float32)
                            nc.vector.tensor_mul(d3[:], d1[:], d2[:])
                            rg = tiny_pool.tile([128, 1], dt.float32)
                            nc.vector.reciprocal(rg[:], d3[:])
                            g = tiny_pool.tile([128, 1], dt.float32)
                            nc.vector.tensor_mul(g[:], nn[:], rg[:])

                            # agreement: dt[n] = sum_d u[n,d]*s[d]; b += g*dt
                            sb16 = tiny_pool.tile([128, CAP_DIM], dt.bfloat16)
                            nc.vector.tensor_copy(sb16[:], s_v[:])
                            tmp2 = tmp_pool.tile([128, N_ROUTE, CAP_DIM], dt.bfloat16)
                            mul_eng.tensor_tensor(
                                tmp2[:], uh_nd, _bcast_mid(sb16[:], N_ROUTE),
                                op=OP.mult,
                            )
                            dta = sm_pool.tile([128, N_ROUTE], dt.float32)
                            nc.vector.tensor_reduce(dta[:], tmp2[:], axis=AX.X, op=OP.add)
                            bnew = sm_pool.tile([128, N_ROUTE], dt.float32)
                            if r == 0:
                                nc.vector.tensor_scalar_mul(bnew[:], dta[:], g[:])
                            else:
                                nc.vector.scalar_tensor_tensor(
                                    bnew[:], dta[:], g[:], b_log[:],
                                    op0=OP.mult, op1=OP.add,
                                )
                            b_log = bnew
                        else:
                            # out = nn/(1+nn) * n1/(n1+1e-8)
                            d1 = tiny_pool.tile([128, 1], dt.float32)
                            nc.vector.tensor_scalar_add(d1[:], nn[:], 1.0)
                            r1 = tiny_pool.tile([128, 1], dt.float32)
                            nc.vector.reciprocal(r1[:], d1[:])
                            d2 = tiny_pool.tile([128, 1], dt.float32)
                            nc.vector.tensor_scalar_add(d2[:], n1[:], 1e-8)
                            r2 = tiny_pool.tile([128, 1], dt.float32)
                            nc.vector.reciprocal(r2[:], d2[:])
                            p1 = tiny_pool.tile([128, 1], dt.float32)
                            nc.vector.tensor_mul(p1[:], nn[:], r1[:])
                            p2 = tiny_pool.tile([128, 1], dt.float32)
                            nc.vector.tensor_mul(p2[:], n1[:], r2[:])
                            nc.vector.tensor_mul(
                                out_sb[:, ch : ch + 1], p1[:], p2[:]
                            )

            nc.sync.dma_start(
                out=out_ap.rearrange("(c p) -> p c", p=128), in_=out_sb[:]
            )

    nc.compile()
    return nc


def _prep_params(W, b_lin, out_caps, hidden=HIDDEN):
    NH = hidden // 128
    w_bf = np.ascontiguousarray(
        W.reshape(NH, 128, NUM_CAPS * CAP_DIM)
    ).astype(BF16)
    caps_bd = np.zeros((NUM_OBJ, 128, 128), np.float32)
    for o in range(NUM_OBJ):
        for i in range(NUM_CAPS):
            caps_bd[o, i * CAP_DIM : (i + 1) * CAP_DIM, i * CAP_DIM : (i + 1) * CAP_DIM] = (
                out_caps[o, i]
            )
    caps_bf = caps_bd.astype(BF16)
    bias_col = np.ascontiguousarray(b_lin.astype(np.float32).reshape(128, 1))
    return w_bf, caps_bf, bias_col


_NC_CACHE = {}


def kernel(x, W, b_lin, out_caps):
    global LAST_EXEC_TIME_NS
    from concourse.bass_utils import run_bass_kernel_spmd

    x = np.asarray(x)
    W = np.asarray(W)
    b_lin = np.asarray(b_lin)
    out_caps = np.asarray(out_caps)
    bsz, hidden = x.shape
    b_sh = bsz // N_CORES

    key = (hidden, b_sh)
    if key not in _NC_CACHE:
        _NC_CACHE[key] = build_bass(hidden=hidden, b_sh=b_sh)
    nc = _NC_CACHE[key]

    w_bf, caps_bf, bias_col = _prep_params(W, b_lin, out_caps, hidden)

    in_maps = []
    for i in range(N_CORES):
        shard = x[i * b_sh : (i + 1) * b_sh]
        xt = np.ascontiguousarray(shard.T)  # [hidden, b_sh]
        in_maps.append(
            {"xt": xt, "w": w_bf, "caps": caps_bf, "bias": bias_col}
        )

    res = run_bass_kernel_spmd(
        nc, in_maps, core_ids=list(range(N_CORES)),
        trace=bool(int(os.environ.get("BASS_TRACE", "0") or "0")),
    )
    LAST_EXEC_TIME_NS = res.exec_time_ns
    return np.concatenate([res.results[i]["out"] for i in range(N_CORES)])
